# revision 24
# baseline (speedup 1.0000x reference)
"""Bahdanau-style attention kernel for Trainium2, data-parallel over batch on 8 cores.

Math (per batch row b):
    h_proj = hidden @ Wh.T + b_attn                      [128]
    energy[s, :] = tanh(h_proj + embs[s] @ We.T)         [S, 128]
    att[s] = v . energy[s, :]                            [S]
    out = softmax_S(where(mask==0, -1e10, att))

Device strategy per core (8 batch rows, S=4096):
  - Host preps layout only: batch shard + transpose of seq_embs to [b, e, s]
    contiguous so DMA streams at full rate and the PE gets the contraction
    dim (e) on partitions directly.
  - PE: We-matmuls [e,128]x[e,512] -> e_projT [d, s] in PSUM;
    one-hot-column v-matmuls contract d and scatter each (b, quarter) att row
    into a persistent [32, 1024] PSUM accumulator (partition = 4*b + s//1024).
  - ACT: tanh with per-partition bias h_projT[:, b]; later exp.
  - DVE: fused mask-multiply + row-sum, reciprocal, final scale.
  - Softmax skips max-subtraction: |att| <= ||v||_1 ~ 5.7 so exp is safe;
    masking is exp(att) * mask which matches where(mask==0,-1e10) exactly.
"""

import numpy as np

B = 64
S = 4096
D = 128  # dec_dim == emb_dim == 128
NCORES = 8
BPC = B // NCORES  # 8 batch rows per core
NQ = 4  # S quarters per batch row
P_ATT = BPC * NQ  # 32 partitions in att accumulator
FQ = S // NQ  # 1024 free elems per quarter

_COMPILED = {}


def _build_bass():
    import concourse.bacc as bacc
    import concourse.mybir as mybir
    from concourse.tile import TileContext

    f32 = mybir.dt.float32
    f32r = mybir.dt.float32r
    bf16 = mybir.dt.bfloat16
    fp16 = mybir.dt.float16
    AF = mybir.ActivationFunctionType
    ALU = mybir.AluOpType

    nc = bacc.Bacc(
        "TRN2", target_bir_lowering=False, debug=False, num_devices=NCORES
    )

    embsT = nc.dram_tensor("embsT", [BPC, D, S], f32, kind="ExternalInput")
    WeT = nc.dram_tensor("WeT", [D, D], fp16, kind="ExternalInput")
    vstrip = nc.dram_tensor("vstrip", [D, 2 * P_ATT], fp16, kind="ExternalInput")
    out_d = nc.dram_tensor("out", [P_ATT, FQ], f32, kind="ExternalOutput")

    with TileContext(nc) as tc:
        with (
            tc.tile_pool(name="consts", bufs=1) as consts,
            tc.tile_pool(name="embs", bufs=4) as embs_pool,
            tc.tile_pool(name="embs16", bufs=4) as embs16_pool,
            tc.tile_pool(name="energy", bufs=9) as energy_pool,
            tc.tile_pool(name="post", bufs=1) as post,
            tc.tile_pool(name="ps_big", bufs=3, space="PSUM") as ps_big,
            tc.tile_pool(name="ps_att", bufs=1, space="PSUM") as ps_att,
        ):
            ps_small = ps_big
            # First embs tiles go out before anything else so the big stream
            # starts at t=0; small consts ride behind in two packed DMAs.
            HW0 = S // 2
            et00 = embs_pool.tile([D, HW0], f32)
            nc.sync.dma_start(out=et00, in_=embsT[0, :, 0:HW0])
            et01 = embs_pool.tile([D, HW0], f32)
            nc.sync.dma_start(out=et01, in_=embsT[0, :, HW0:S])

            WeT_sb = consts.tile([D, D], fp16)
            nc.sync.dma_start(out=WeT_sb, in_=WeT[:, :])
            vstrip_sb = consts.tile([D, 2 * P_ATT], fp16)
            nc.sync.dma_start(out=vstrip_sb, in_=vstrip[:, :])
            CPW = FQ + D + BPC + 1 + BPC + P_ATT  # 1024+128+8+1+8+32
            cpack = nc.dram_tensor("cpack", [D, CPW], f32, kind="ExternalInput")
            cpack_sb = consts.tile([D, CPW], f32)
            nc.sync.dma_start(out=cpack_sb, in_=cpack[:, :])
            o = FQ
            maskT_sb = cpack_sb[0:P_ATT, 0:FQ]
            WhT_sb = cpack_sb[:, o : o + D]; o += D
            hiddenT_sb = cpack_sb[:, o : o + BPC]; o += BPC
            b_attn_sb = cpack_sb[:, o : o + 1]; o += 1
            qsel_sb = cpack_sb[0:P_ATT, o : o + BPC]; o += BPC
            qselT_sb = cpack_sb[0:BPC, o : o + P_ATT]

            # h_projT[d, b] = sum_k WhT[k, d] * hiddenT[k, b] + b_attn[d]
            hp_ps = ps_small.tile([D, BPC], f32, tag="ps")
            nc.tensor.matmul(hp_ps[:, :], WhT_sb[:, :], hiddenT_sb[:, :])
            hprojT_sb = consts.tile([D, BPC], f32)
            nc.vector.tensor_scalar_add(hprojT_sb[:, :], hp_ps[:, :], b_attn_sb[:, 0:1])

            # att accumulator [32, 1024]: partition 4*b + q, free = s % 1024
            att_ps = ps_att.tile([P_ATT, FQ], f32)

            # Software pipeline over b: PE does all 8 We-matmuls of batch b
            # back-to-back (shared stationary, HAM stays warm), then the
            # v-matmuls of batch b-1 whose tanh outputs are long since ready.
            HW = S // 2  # 2048-wide DMA halves
            n_vmm = 0

            def emit_vmms(pending):
                nonlocal n_vmm
                for en_t, p in pending:
                    for m in range(2):
                        nc.tensor.matmul(
                            att_ps[:, 512 * m : 512 * (m + 1)],
                            vstrip_sb[:, P_ATT - p : 2 * P_ATT - p],
                            en_t[:, 512 * m : 512 * (m + 1)],
                            start=(n_vmm < 2),
                            stop=(n_vmm >= 2 * NQ * BPC - 2),
                            skip_group_check=True,
                        )
                        n_vmm += 1

            prev = []
            for b in range(BPC):
                cur = []
                for h2 in range(2):
                    if b == 0:
                        etf = et00 if h2 == 0 else et01
                    else:
                        etf = embs_pool.tile([D, HW], f32)
                        nc.sync.dma_start(
                            out=etf, in_=embsT[b, :, h2 * HW : (h2 + 1) * HW]
                        )
                    et = embs16_pool.tile([D, HW], fp16)
                    if (2 * b + h2) % 2 == 0:
                        nc.gpsimd.tensor_copy(et[:, :], etf[:, :])
                    else:
                        nc.vector.tensor_copy(et[:, :], etf[:, :])
                    for j in range(2):
                        q = 2 * h2 + j  # quarter index: s in [1024q, 1024q+1024)
                        pe_t = ps_big.tile([D, FQ], f32, tag="ps")
                        for m in range(2):
                            nc.tensor.matmul(
                                pe_t[:, 512 * m : 512 * (m + 1)],
                                WeT_sb[:, :],
                                et[:, 1024 * j + 512 * m : 1024 * j + 512 * (m + 1)],
                            )
                        en_t = energy_pool.tile([D, FQ], fp16)
                        nc.scalar.activation(
                            out=en_t[:, :],
                            in_=pe_t[:, :],
                            func=AF.Tanh,
                            bias=hprojT_sb[:, b : b + 1],
                            scale=1.0,
                        )
                        # one-hot column p of the vstrip window routes
                        # v.energy into att_ps partition p = 4*b + q
                        cur.append((en_t, NQ * b + q))
                emit_vmms(prev)
                prev = cur
            emit_vmms(prev)

            # softmax over s (per batch row): p = exp(att) * mask
            p_sb = post.tile([P_ATT, FQ], f32)
            nc.scalar.activation(out=p_sb[:, :], in_=att_ps[:, :], func=AF.Exp)
            pm_sb = post.tile([P_ATT, FQ], f32)
            partials_sb = post.tile([P_ATT, 1], f32)
            nc.vector.tensor_mul(pm_sb[:, :], p_sb[:, :], maskT_sb[:, :])
            nc.vector.reduce_sum(
                out=partials_sb[:, 0:1], in_=pm_sb[:, :], axis=mybir.AxisListType.X
            )
            # denom[b] = sum over the 4 quarter-partials of batch b
            den_ps = ps_small.tile([BPC, 1], f32, tag="ps")
            nc.tensor.matmul(den_ps[:, :], qsel_sb[:, :], partials_sb[:, 0:1])
            recip8_sb = post.tile([BPC, 1], f32)
            nc.vector.reciprocal(recip8_sb[:, :], den_ps[:, :])
            # spread 1/denom back to the 32-partition layout
            r32_ps = ps_small.tile([P_ATT, 1], f32, tag="ps")
            nc.tensor.matmul(r32_ps[:, :], qselT_sb[:, :], recip8_sb[:, 0:1])
            recip32_sb = post.tile([P_ATT, 1], f32)
            nc.vector.tensor_copy(recip32_sb[:, :], r32_ps[:, :])

            out_sb = post.tile([P_ATT, FQ], f32)
            nc.vector.tensor_scalar_mul(out_sb[:, :], pm_sb[:, :], recip32_sb[:, 0:1])
            nc.sync.dma_start(out=out_d[:, :], in_=out_sb[:, :])

    nc.compile()
    return nc


def _get_nc():
    if "nc" not in _COMPILED:
        _COMPILED["nc"] = _build_bass()
    return _COMPILED["nc"]


def _prep_inputs(hidden, seq_embs, mask, W_attn, b_attn, v_w):
    """Host-side layout-only prep: shard over batch + relayouts (no math)."""
    hidden = np.asarray(hidden, dtype=np.float32)
    seq_embs = np.asarray(seq_embs, dtype=np.float32)
    mask = np.asarray(mask)
    W_attn = np.asarray(W_attn, dtype=np.float32)
    b_attn = np.asarray(b_attn, dtype=np.float32)
    v_w = np.asarray(v_w, dtype=np.float32)

    WhT = np.ascontiguousarray(W_attn[:, :D].T)
    WeT = np.ascontiguousarray(W_attn[:, D:].T.astype(np.float16))
    b_col = np.ascontiguousarray(b_attn.reshape(D, 1))
    vstrip = np.zeros((D, 2 * P_ATT), dtype=np.float16)
    vstrip[:, P_ATT] = v_w[0].astype(np.float16)
    qsel = np.zeros((P_ATT, BPC), dtype=np.float32)
    for p in range(P_ATT):
        qsel[p, p // NQ] = 1.0
    qselT = np.ascontiguousarray(qsel.T)

    in_maps = []
    for c in range(NCORES):
        bsl = slice(c * BPC, (c + 1) * BPC)
        embsT = np.ascontiguousarray(
            np.transpose(seq_embs[:, bsl, :], (1, 2, 0))
        )  # [8, 128, 4096]
        maskT = np.ascontiguousarray(
            mask[bsl].astype(np.float32).reshape(P_ATT, FQ)
        )
        hiddenT = np.ascontiguousarray(hidden[bsl].T)  # [128, 8]
        CPW = FQ + D + BPC + 1 + BPC + P_ATT
        cpack = np.zeros((D, CPW), dtype=np.float32)
        cpack[:P_ATT, :FQ] = maskT
        o = FQ
        cpack[:, o : o + D] = WhT; o += D
        cpack[:, o : o + BPC] = hiddenT; o += BPC
        cpack[:, o : o + 1] = b_col; o += 1
        cpack[:P_ATT, o : o + BPC] = qsel; o += BPC
        cpack[:BPC, o : o + P_ATT] = qselT
        in_maps.append(
            {
                "embsT": embsT,
                "WeT": WeT,
                "vstrip": vstrip,
                "cpack": cpack,
            }
        )
    return in_maps


def kernel(hidden, seq_embs, mask, W_attn, b_attn, v_w, **run_kwargs):
    from concourse.bass_utils import run_bass_kernel_spmd

    nc = _get_nc()
    in_maps = _prep_inputs(hidden, seq_embs, mask, W_attn, b_attn, v_w)
    res = run_bass_kernel_spmd(
        nc, in_maps, core_ids=list(range(NCORES)), **run_kwargs
    )
    out = np.concatenate(
        [r["out"].reshape(BPC, S) for r in res.results], axis=0
    ).astype(np.float32)
    if run_kwargs:
        kernel.last_results = res  # stash for the profiling harness
    return out


# revision 25
# speedup vs baseline: 1.1902x; 1.1902x over previous
"""Bahdanau-style attention kernel for Trainium2, data-parallel over batch on 8 cores.

Math (per batch row b):
    h_proj = hidden @ Wh.T + b_attn                      [128]
    energy[s, :] = tanh(h_proj + embs[s] @ We.T)         [S, 128]
    att[s] = v . energy[s, :]                            [S]
    out = softmax_S(where(mask==0, -1e10, att))

Device strategy per core (8 batch rows, S=4096):
  - Host preps layout only: batch shard + transpose of seq_embs to [b, e, s]
    contiguous so DMA streams at full rate and the PE gets the contraction
    dim (e) on partitions directly.
  - PE: We-matmuls [e,128]x[e,512] -> e_projT [d, s] in PSUM;
    one-hot-column v-matmuls contract d and scatter each (b, quarter) att row
    into a persistent [32, 1024] PSUM accumulator (partition = 4*b + s//1024).
  - ACT: tanh with per-partition bias h_projT[:, b]; later exp.
  - DVE: fused mask-multiply + row-sum, reciprocal, final scale.
  - Softmax skips max-subtraction: |att| <= ||v||_1 ~ 5.7 so exp is safe;
    masking is exp(att) * mask which matches where(mask==0,-1e10) exactly.
"""

import numpy as np

B = 64
S = 4096
D = 128  # dec_dim == emb_dim == 128
NCORES = 8
BPC = B // NCORES  # 8 batch rows per core
NQ = 4  # S quarters per batch row
P_ATT = BPC * NQ  # 32 partitions in att accumulator
FQ = S // NQ  # 1024 free elems per quarter

_COMPILED = {}


def _build_bass():
    import concourse.bacc as bacc
    import concourse.mybir as mybir
    from concourse.tile import TileContext

    f32 = mybir.dt.float32
    f32r = mybir.dt.float32r
    bf16 = mybir.dt.bfloat16
    fp16 = mybir.dt.float16
    AF = mybir.ActivationFunctionType
    ALU = mybir.AluOpType

    nc = bacc.Bacc(
        "TRN2", target_bir_lowering=False, debug=False, num_devices=NCORES
    )

    embsT = nc.dram_tensor("embsT", [BPC, D, S], f32, kind="ExternalInput")
    WeT = nc.dram_tensor("WeT", [D, D], fp16, kind="ExternalInput")
    vstrip = nc.dram_tensor("vstrip", [D, 2 * P_ATT], fp16, kind="ExternalInput")
    out_d = nc.dram_tensor("out", [P_ATT, FQ], f32, kind="ExternalOutput")

    with TileContext(nc) as tc:
        with (
            tc.tile_pool(name="consts", bufs=1) as consts,
            tc.tile_pool(name="embs", bufs=4) as embs_pool,
            tc.tile_pool(name="embs16", bufs=4) as embs16_pool,
            tc.tile_pool(name="energy", bufs=9) as energy_pool,
            tc.tile_pool(name="post", bufs=1) as post,
            tc.tile_pool(name="ps_big", bufs=3, space="PSUM") as ps_big,
            tc.tile_pool(name="ps_att", bufs=1, space="PSUM") as ps_att,
        ):
            ps_small = ps_big
            # First embs tiles go out before anything else so the big stream
            # starts at t=0; small consts ride behind in two packed DMAs.
            HW0 = S // 2
            et00 = embs_pool.tile([D, HW0], f32)
            nc.sync.dma_start(out=et00, in_=embsT[0, :, 0:HW0])
            et01 = embs_pool.tile([D, HW0], f32)
            nc.sync.dma_start(out=et01, in_=embsT[0, :, HW0:S])

            WeT_sb = consts.tile([D, D], fp16)
            nc.sync.dma_start(out=WeT_sb, in_=WeT[:, :])
            vstrip_sb = consts.tile([D, 2 * P_ATT], fp16)
            nc.sync.dma_start(out=vstrip_sb, in_=vstrip[:, :])
            CPW = FQ + D + BPC + 1 + BPC + P_ATT  # 1024+128+8+1+8+32
            cpack = nc.dram_tensor("cpack", [D, CPW], f32, kind="ExternalInput")
            cpack_sb = consts.tile([D, CPW], f32)
            nc.sync.dma_start(out=cpack_sb, in_=cpack[:, :])
            o = FQ
            maskT_sb = cpack_sb[0:P_ATT, 0:FQ]
            WhT_sb = cpack_sb[:, o : o + D]; o += D
            hiddenT_sb = cpack_sb[:, o : o + BPC]; o += BPC
            b_attn_sb = cpack_sb[:, o : o + 1]; o += 1
            qsel_sb = cpack_sb[0:P_ATT, o : o + BPC]; o += BPC
            qselT_sb = cpack_sb[0:BPC, o : o + P_ATT]

            # h_projT[d, b] = sum_k WhT[k, d] * hiddenT[k, b] + b_attn[d]
            hp_ps = ps_small.tile([D, BPC], f32, tag="ps")
            nc.tensor.matmul(hp_ps[:, :], WhT_sb[:, :], hiddenT_sb[:, :])
            hprojT_sb = consts.tile([D, BPC], f32)
            nc.vector.tensor_scalar_add(hprojT_sb[:, :], hp_ps[:, :], b_attn_sb[:, 0:1])

            # att accumulator [32, 1024]: partition 4*b + q, free = s % 1024
            att_ps = ps_att.tile([P_ATT, FQ], f32)

            # Software pipeline over b: PE does all 8 We-matmuls of batch b
            # back-to-back (shared stationary, HAM stays warm), then the
            # v-matmuls of batch b-1 whose tanh outputs are long since ready.
            HW = S // 2  # 2048-wide DMA halves
            n_vmm = 0

            def emit_vmms(pending):
                nonlocal n_vmm
                for en_t, p in pending:
                    for m in range(2):
                        nc.tensor.matmul(
                            att_ps[:, 512 * m : 512 * (m + 1)],
                            vstrip_sb[:, P_ATT - p : 2 * P_ATT - p],
                            en_t[:, 512 * m : 512 * (m + 1)],
                            start=(n_vmm < 2),
                            stop=(n_vmm >= 2 * NQ * BPC - 2),
                            skip_group_check=True,
                        )
                        n_vmm += 1

            prev = []
            for b in range(BPC):
                cur = []
                for h2 in range(2):
                    if b == 0:
                        etf = et00 if h2 == 0 else et01
                    else:
                        etf = embs_pool.tile([D, HW], f32)
                        nc.sync.dma_start(
                            out=etf, in_=embsT[b, :, h2 * HW : (h2 + 1) * HW]
                        )
                    et = embs16_pool.tile([D, HW], fp16)
                    nc.vector.tensor_copy(et[:, :], etf[:, :])
                    for j in range(2):
                        q = 2 * h2 + j  # quarter index: s in [1024q, 1024q+1024)
                        pe_t = ps_big.tile([D, FQ], f32, tag="ps")
                        for m in range(2):
                            nc.tensor.matmul(
                                pe_t[:, 512 * m : 512 * (m + 1)],
                                WeT_sb[:, :],
                                et[:, 1024 * j + 512 * m : 1024 * j + 512 * (m + 1)],
                            )
                        en_t = energy_pool.tile([D, FQ], fp16)
                        nc.scalar.activation(
                            out=en_t[:, :],
                            in_=pe_t[:, :],
                            func=AF.Tanh,
                            bias=hprojT_sb[:, b : b + 1],
                            scale=1.0,
                        )
                        # one-hot column p of the vstrip window routes
                        # v.energy into att_ps partition p = 4*b + q
                        cur.append((en_t, NQ * b + q))
                emit_vmms(prev)
                prev = cur
            emit_vmms(prev)

            # softmax over s (per batch row): p = exp(att + maskbias),
            # maskbias = -30 on masked positions (exp -> ~1e-16, matches the
            # reference's exact zeros to float precision); accum_out gives the
            # row-sums in the same ACT pass.
            att_m = post.tile([P_ATT, FQ], f32)
            nc.vector.tensor_add(att_m[:, :], att_ps[:, :], maskT_sb[:, :])
            p_sb = post.tile([P_ATT, FQ], f32)
            partials_sb = post.tile([P_ATT, 1], f32)
            nc.scalar.activation(
                out=p_sb[:, :],
                in_=att_m[:, :],
                func=AF.Exp,
                accum_out=partials_sb[:, 0:1],
            )
            # denom[b] = sum over the 4 quarter-partials of batch b
            den_ps = ps_small.tile([BPC, 1], f32, tag="ps")
            nc.tensor.matmul(den_ps[:, :], qsel_sb[:, :], partials_sb[:, 0:1])
            recip8_sb = post.tile([BPC, 1], f32)
            nc.vector.reciprocal(recip8_sb[:, :], den_ps[:, :])
            # spread 1/denom back to the 32-partition layout
            r32_ps = ps_small.tile([P_ATT, 1], f32, tag="ps")
            nc.tensor.matmul(r32_ps[:, :], qselT_sb[:, :], recip8_sb[:, 0:1])
            recip32_sb = post.tile([P_ATT, 1], f32)
            nc.vector.tensor_copy(recip32_sb[:, :], r32_ps[:, :])

            out_sb = post.tile([P_ATT, FQ], f32)
            nc.vector.tensor_scalar_mul(out_sb[:, :], p_sb[:, :], recip32_sb[:, 0:1])
            nc.sync.dma_start(out=out_d[:, :], in_=out_sb[:, :])

    nc.compile()
    return nc


def _get_nc():
    if "nc" not in _COMPILED:
        _COMPILED["nc"] = _build_bass()
    return _COMPILED["nc"]


def _prep_inputs(hidden, seq_embs, mask, W_attn, b_attn, v_w):
    """Host-side layout-only prep: shard over batch + relayouts (no math)."""
    hidden = np.asarray(hidden, dtype=np.float32)
    seq_embs = np.asarray(seq_embs, dtype=np.float32)
    mask = np.asarray(mask)
    W_attn = np.asarray(W_attn, dtype=np.float32)
    b_attn = np.asarray(b_attn, dtype=np.float32)
    v_w = np.asarray(v_w, dtype=np.float32)

    WhT = np.ascontiguousarray(W_attn[:, :D].T)
    WeT = np.ascontiguousarray(W_attn[:, D:].T.astype(np.float16))
    b_col = np.ascontiguousarray(b_attn.reshape(D, 1))
    vstrip = np.zeros((D, 2 * P_ATT), dtype=np.float16)
    vstrip[:, P_ATT] = v_w[0].astype(np.float16)
    qsel = np.zeros((P_ATT, BPC), dtype=np.float32)
    for p in range(P_ATT):
        qsel[p, p // NQ] = 1.0
    qselT = np.ascontiguousarray(qsel.T)

    in_maps = []
    for c in range(NCORES):
        bsl = slice(c * BPC, (c + 1) * BPC)
        embsT = np.ascontiguousarray(
            np.transpose(seq_embs[:, bsl, :], (1, 2, 0))
        )  # [8, 128, 4096]
        maskT = np.ascontiguousarray(
            (mask[bsl].astype(np.float32).reshape(P_ATT, FQ) - 1.0) * 30.0
        )
        hiddenT = np.ascontiguousarray(hidden[bsl].T)  # [128, 8]
        CPW = FQ + D + BPC + 1 + BPC + P_ATT
        cpack = np.zeros((D, CPW), dtype=np.float32)
        cpack[:P_ATT, :FQ] = maskT
        o = FQ
        cpack[:, o : o + D] = WhT; o += D
        cpack[:, o : o + BPC] = hiddenT; o += BPC
        cpack[:, o : o + 1] = b_col; o += 1
        cpack[:P_ATT, o : o + BPC] = qsel; o += BPC
        cpack[:BPC, o : o + P_ATT] = qselT
        in_maps.append(
            {
                "embsT": embsT,
                "WeT": WeT,
                "vstrip": vstrip,
                "cpack": cpack,
            }
        )
    return in_maps


def kernel(hidden, seq_embs, mask, W_attn, b_attn, v_w, **run_kwargs):
    from concourse.bass_utils import run_bass_kernel_spmd

    nc = _get_nc()
    in_maps = _prep_inputs(hidden, seq_embs, mask, W_attn, b_attn, v_w)
    res = run_bass_kernel_spmd(
        nc, in_maps, core_ids=list(range(NCORES)), **run_kwargs
    )
    out = np.concatenate(
        [r["out"].reshape(BPC, S) for r in res.results], axis=0
    ).astype(np.float32)
    if run_kwargs:
        kernel.last_results = res  # stash for the profiling harness
    return out
